# revision 5
# baseline (speedup 1.0000x reference)
"""Trainium2 Bass kernel v2 for BlockDiagonalAggregator (moe_routing).

Computes, for each batch row b:
    logit[b,k] = dot(keys[sigma[b,k]], h[b,k,:])   (masked -inf where sigma==64)
    alpha      = softmax_k(logit)
    out[b,:]   = sum_k alpha[b,k] * h[b,k,:]

Distribution: data-parallel over B across 8 NeuronCores (512 rows each),
keys replicated, no collectives.

v2 changes vs v1 (~257us -> ~100us measured):
  - h, oh, keys, E all bf16 (halves the dominant h DMA: 67MB -> 33.5MB/core).
  - penalty applied by one per-macro DVE add on the (128,16) logit tile
    (tensor_tensor_reduce's initial-value-AP path faults on HW; stt +
    separate add is equally fast).
  - exp batched: 2 ACT ops per macro (stride-34 column AP into the packed
    E tile) instead of 2 ops per chunk (512 tiny ACT ops -> 32).
  - E stationaries packed into one (128, 512) tile per macro (x2 buffers),
    zero cells memset once; chunk c's stationary = E[:, 32c:32c+32] with
    nonzero cols 2c, 2c+1 (rows 0:64 / 64:128).
  - oh/keys at SBUF partitions 64:128 (LDWEIGHTS requires 64-partition
    stationaries to start at partition 0 or 64).
  - oh DMA issued from the SP queue: on the ACT queue it sat behind
    exp/scale ops whose semaphore waits blocked each macro's oh prefetch
    (~3us/macro head-of-line stall).
  - out scale moved to ACT (Copy with per-partition scale AP), freeing DVE.
  - 6-deep w PSUM rotation (psw=6/psp=1/pse=1): with only 4 banks the
    in-order PE queue blocked at w-matmuls waiting for the DVE logit dot
    to release banks, dragging pool matmuls with it (~45us).
  - macro m's w-matmuls interleaved per-chunk with macro m-1's pool
    matmuls so the in-order PE queue keeps feeding the DVE logit chain
    (the critical path) while pooling proceeds one macro behind.

Per-core algorithm (single streaming pass over h):
  chunk = 128 (b,k)-slots = 2 batch rows; macro = 16 chunks = 32 b-rows.
  w gather:  PE  w_ps = oh_c.T @ keys            (bf16 x bf16 -> f32 PSUM)
  logit:     DVE tensor_tensor_reduce(h_c * w_ps, init=pen) -> logit[:,c]
  e:         ACT exp(logit) -> E block cells (bf16)
  pool/esum: PE  pool += E_c.T @ h_c ; esum += E_c.T @ ones   (PSUM accum)
  out:       DVE reciprocal(esum); ACT pool * (1/esum) -> DMA out.
"""

import numpy as np
import ml_dtypes

# Problem constants (hardcoded: kernel.py must be self-contained)
B, K, D = 4096, 64, 512
N_AGENTS = 64
N_CORES = 8
B_CORE = B // N_CORES            # 512
BK_CORE = B_CORE * K             # 32768
CHUNK = 128                      # bk-slots per chunk (= 2 batch rows)
CHUNKS_PER_MACRO = 16            # chunks per macro (= 32 batch rows)
MACRO_BK = CHUNK * CHUNKS_PER_MACRO   # 2048
NEG_BIG = -1e9

_prog_cache = {}


def _build_program(n_macros: int, repeat: int = 1, probe: str = ""):
    """Build the SPMD single-core Bass program for a shard of
    n_macros * MACRO_BK (b,k)-slots. repeat>1 wraps the macro loop in a
    device-side For doing the identical (idempotent) work `repeat` times
    (timing only)."""
    import contextlib
    import concourse.bacc as bacc
    import concourse.tile as tile
    import concourse.mybir as mybir

    f32 = mybir.dt.float32
    bf16 = mybir.dt.bfloat16
    AF = mybir.ActivationFunctionType
    ALU = mybir.AluOpType

    CM = CHUNKS_PER_MACRO
    b_rows = n_macros * MACRO_BK // K
    RPM = MACRO_BK // K   # 32 output rows per macro
    half = CHUNK // 2     # 64 = K

    nc = bacc.Bacc("TRN2", target_bir_lowering=False, debug=False,
                   num_devices=N_CORES)

    h_d = nc.dram_tensor("h", [n_macros, CHUNK, CM * D], bf16,
                         kind="ExternalInput").ap()
    oh_d = nc.dram_tensor("oh", [n_macros, N_AGENTS, CM, CHUNK],
                          bf16, kind="ExternalInput").ap()
    pen_d = nc.dram_tensor("pen", [CHUNK, n_macros * CM], f32,
                           kind="ExternalInput").ap()
    keys_d = nc.dram_tensor("keys", [N_AGENTS, D], bf16,
                            kind="ExternalInput").ap()
    out_d = nc.dram_tensor("out", [b_rows, D], f32, kind="ExternalOutput").ap()

    with tile.TileContext(nc) as tc:
        with (
            tc.tile_pool(name="const", bufs=1) as const_pool,
            tc.tile_pool(name="h", bufs=6) as h_pool,
            tc.tile_pool(name="oh", bufs=4) as oh_pool,
            tc.tile_pool(name="tmp", bufs=4) as tmp_pool,
            tc.tile_pool(name="logit", bufs=3) as logit_pool,
            tc.tile_pool(name="outp", bufs=3) as out_pool,
            tc.tile_pool(name="recip", bufs=2) as recip_pool,
            tc.tile_pool(name="psw", bufs=6, space="PSUM") as psw,
            tc.tile_pool(name="psp", bufs=1, space="PSUM") as psp,
            tc.tile_pool(name="pse", bufs=1, space="PSUM") as pse,
        ):
            # keys and oh live at partitions 64:128 (LDWEIGHTS needs a
            # 64-partition stationary to start at partition 0 or 64)
            OHP = 64
            keys_t = const_pool.tile([CHUNK, D], bf16)
            nc.sync.dma_start(keys_t[OHP:OHP + N_AGENTS, :], keys_d[:])
            pen_t = const_pool.tile([CHUNK, n_macros * CM], f32)
            nc.scalar.dma_start(pen_t[:], pen_d[:])
            ones_t = const_pool.tile([CHUNK, 2], bf16)
            nc.vector.memset(ones_t[:], 1.0)

            # packed per-macro E stationaries (x2 alternating); chunk c's
            # slice is cols 32c:32c+32 with nonzero cols 34c (rows 0:64) and
            # 34c+1 (rows 64:128), rewritten by ACT every other macro; the
            # zero cells stay zero forever after this one-time memset
            E_bufs = []
            for i in range(2):
                e = const_pool.tile([CHUNK, CM * RPM], bf16,
                                    tag=f"e_all_{i}")
                nc.vector.memset(e[:], 0.0)
                E_bufs.append(e)

            wconst_ps = None
            if probe == "nope":
                wconst_ps = psw.tile([CHUNK, D], f32, tag="wconst")
                nc.vector.memset(wconst_ps[:], 0.001)

            hd3 = h_d.rearrange("m p (c d) -> m p c d", d=D)

            def emit_dma(m):
                """h/oh loads for macro m."""
                h_t = h_pool.tile([CHUNK, CM, D], bf16)
                hc = CM // 2
                nc.sync.dma_start(h_t[:, 0:hc, :], hd3[m][:, 0:hc, :])
                nc.sync.dma_start(h_t[:, hc:, :], hd3[m][:, hc:, :])
                oh_t = oh_pool.tile([CHUNK, CM, CHUNK], bf16)
                nc.sync.dma_start(oh_t[OHP:OHP + N_AGENTS], oh_d[m])
                if probe == "dma":
                    out_t = out_pool.tile([RPM, D], f32)
                    nc.vector.tensor_copy(out_t[:], h_t[0:RPM, 0, :])
                    nc.scalar.dma_start(out_d[m * RPM:(m + 1) * RPM, :],
                                        out_t[:])
                return h_t, oh_t

            def emit_w_logit(m, c, oh_t, h_t, logit_t):
                """w gather matmul + fused logit dot for (m, c)."""
                if probe in ("nope",):
                    w_ps = wconst_ps
                else:
                    w_ps = psw.tile([CHUNK, D], f32)
                    nc.tensor.matmul(w_ps[:],
                                     oh_t[OHP:OHP + N_AGENTS, c, :],
                                     keys_t[OHP:OHP + N_AGENTS, :],
                                     start=True, stop=True)
                if probe == "nodve":
                    return
                tmp_t = tmp_pool.tile([CHUNK, D], bf16)
                nc.vector.scalar_tensor_tensor(
                    tmp_t[:], h_t[:, c, :], 1.0, w_ps[:],
                    op0=ALU.mult, op1=ALU.mult,
                    accum_out=logit_t[:, c:c + 1])

            def emit_exp(m, logit_t):
                """mask + batched exp into macro m's packed E tile."""
                E = E_bufs[m % 2]
                # mask: logit += pen (-1e9 on unassigned slots), one DVE op
                logit_m = logit_pool.tile([CHUNK, CM], f32, tag="logit_m")
                nc.vector.tensor_add(logit_m[:], logit_t[:],
                                     pen_t[:, m * CM:(m + 1) * CM])
                # e = exp(logit) into packed block columns (stride 34)
                nc.scalar.activation(E[0:half, 0:CM * RPM:RPM + 2],
                                     logit_m[0:half, :], AF.Exp)
                nc.scalar.activation(E[half:CHUNK, 1:CM * RPM:RPM + 2],
                                     logit_m[half:CHUNK, :], AF.Exp)

            def emit_pool(m, c, h_t, pool_ps, esum_ps):
                """one chunk of pool/esum accumulation for macro m."""
                first, last = (c == 0), (c == CM - 1)
                E = E_bufs[m % 2]
                Ec = E[:, RPM * c:RPM * (c + 1)]
                nc.tensor.matmul(pool_ps[:], Ec, h_t[:, c, :],
                                 start=first, stop=last)
                nc.tensor.matmul(esum_ps[:], Ec, ones_t[:],
                                 start=first, stop=last)

            def emit_store(m, pool_ps, esum_ps):
                """normalize + store macro m."""
                recip_t = recip_pool.tile([RPM, 1], f32)
                nc.vector.reciprocal(recip_t[:], esum_ps[:, 0:1])
                out_t = out_pool.tile([RPM, D], f32)
                nc.scalar.activation(out_t[:], pool_ps[:], AF.Copy,
                                     scale=recip_t[:])
                nc.scalar.dma_start(out_d[m * RPM:(m + 1) * RPM, :], out_t[:])

            def emit_macro(m, prev):
                """front half of macro m, interleaved per-chunk with the
                back half of macro m-1 so the in-order PE queue always has
                a fresh w matmul between pool matmuls (the DVE logit chain
                is the critical path and must never starve)."""
                h_t, oh_t = emit_dma(m)
                if probe == "dma":
                    return (m, h_t)
                back = probe not in ("nope", "wdve") and prev is not None
                if back:
                    pm, ph_t = prev
                    pool_ps = psp.tile([RPM, D], f32)
                    esum_ps = pse.tile([RPM, 2], f32)
                logit_t = logit_pool.tile([CHUNK, CM], f32)
                for c in range(CM):
                    emit_w_logit(m, c, oh_t, h_t, logit_t)
                    if back:
                        emit_pool(pm, c, ph_t, pool_ps, esum_ps)
                if back:
                    emit_store(pm, pool_ps, esum_ps)
                if probe != "nodve":
                    emit_exp(m, logit_t)
                return (m, h_t)

            def emit_tail(prev):
                if probe in ("dma", "nope", "wdve") or prev is None:
                    return
                pm, ph_t = prev
                pool_ps = psp.tile([RPM, D], f32)
                esum_ps = pse.tile([RPM, 2], f32)
                for c in range(CM):
                    emit_pool(pm, c, ph_t, pool_ps, esum_ps)
                emit_store(pm, pool_ps, esum_ps)

            rep_ctx = (tc.For_i(0, repeat, 1) if repeat > 1
                       else contextlib.nullcontext())
            with rep_ctx:
                prev = None
                for m in range(n_macros):
                    prev = emit_macro(m, prev)
                emit_tail(prev)

    nc.compile()
    return nc


def get_program(n_macros: int = B_CORE * K // MACRO_BK, repeat: int = 1):
    key = (n_macros, repeat)
    if key not in _prog_cache:
        _prog_cache[key] = _build_program(n_macros, repeat=repeat)
    return _prog_cache[key]


def prep_core_inputs(h_bk: np.ndarray, sigma_bk: np.ndarray,
                     keys_bf16: np.ndarray):
    """Host-side prep of one core's input map.
    h_bk: (bk, D) float32, sigma_bk: (bk,) int."""
    bk = h_bk.shape[0]
    n_macros = bk // MACRO_BK
    CM = CHUNKS_PER_MACRO
    sig = sigma_bk.astype(np.int64)
    # one-hot (a == sigma); sigma == N_AGENTS (unassigned) matches nothing
    oh = (sig[None, :] == np.arange(N_AGENTS, dtype=np.int64)[:, None])
    oh = oh.astype(ml_dtypes.bfloat16)          # (A, bk)
    oh = oh.reshape(N_AGENTS, n_macros, CM, CHUNK)
    oh = np.ascontiguousarray(oh.transpose(1, 0, 2, 3))  # (m, A, c, j)

    pen = np.where(sig < N_AGENTS, np.float32(0.0), np.float32(NEG_BIG))
    pen = pen.reshape(n_macros, CM, CHUNK)
    pen = np.ascontiguousarray(pen.transpose(2, 0, 1)).astype(np.float32)
    pen = pen.reshape(CHUNK, n_macros * CM)

    h_shuf = np.ascontiguousarray(
        h_bk.reshape(n_macros, CM, CHUNK, D)
            .transpose(0, 2, 1, 3)).astype(ml_dtypes.bfloat16)
    h_shuf = h_shuf.reshape(n_macros, CHUNK, CM * D)

    return {
        "h": h_shuf,
        "oh": oh,
        "pen": pen,
        "keys": keys_bf16,
    }


def kernel(h, keys, sigma):
    from concourse.bass_utils import run_bass_kernel_spmd

    h = np.asarray(h, dtype=np.float32)
    keys = np.asarray(keys, dtype=np.float32)
    sigma = np.asarray(sigma)

    keys_bf16 = keys.astype(ml_dtypes.bfloat16)
    h2 = h.reshape(B * K, D)
    sig2 = sigma.reshape(B * K)

    in_maps = []
    for i in range(N_CORES):
        lo, hi = i * BK_CORE, (i + 1) * BK_CORE
        in_maps.append(prep_core_inputs(h2[lo:hi], sig2[lo:hi], keys_bf16))

    nc = get_program()
    res = run_bass_kernel_spmd(nc, in_maps, list(range(N_CORES)))
    out = np.concatenate([res.results[i]["out"] for i in range(N_CORES)],
                         axis=0)
    return out.astype(np.float32)


if __name__ == "__main__":
    rng = np.random.default_rng(0)
    h = rng.standard_normal((B, K, D), dtype=np.float32)
    keys = (rng.standard_normal((N_AGENTS, D), dtype=np.float32) * 0.01)
    sigma = rng.integers(0, N_AGENTS + 1, size=(B, K)).astype(np.int32)
    out = kernel(h=h, keys=keys, sigma=sigma)
    print("out", out.shape, out.dtype, float(np.abs(out).mean()))
